# revision 1
# baseline (speedup 1.0000x reference)
"""GwcVolumeCostProcessor Trainium2 kernel.

Builds the groupwise-correlation + concat cost volume:
  out[1, 64, 48, 128, 240] f32 from
  ref_gwc/tgt_gwc [1, 320, 128, 240] and ref_concat/tgt_concat [1, 12, 128, 240].

Sharding: H axis (128 = 8 cores x 16 rows). The disparity shift is along W
only, so each core needs just its own 16-row slice of every input.

Per-core pipeline (for each disparity d, descending):
  - DVE: prod[c,h,w] = ref_bf16[c,h,w+d] * tgt_bf16[c,h,w]   (bf16, 2x mode)
  - PE : block-diagonal [128,16] bf16 matmul reduces groups of 8 channels
         (mean) into PSUM
  - ACT: drains PSUM -> staging (fp32)
  - DMA: staging -> DRAM gwc channels; concat channels DMA'd straight from
         SBUF inputs with a zero-buffer for the w<d strips.
Staging slots are fully zeroed once; descending-d order keeps the w<d strip
zero without per-d memsets.
"""

import numpy as np
import ml_dtypes

C = 320          # gwc channels
G = 40           # groups
CPG = 8          # channels per group
D = 48           # disparity bins
H = 128          # full height
W = 240          # width
CC = 12          # concat channels per tensor
COUT = G + 2 * CC  # 64 output channels
NCORES = 8
HS = H // NCORES  # 16 rows per core

# channel tiles on partitions: (start, count, psum_base_partition, out_group_count)
# psum base partitions must be 32-aligned (PE col_grp constraint), so the
# three group blocks land sparsely at psum/staging partitions 0, 32, 64.
CH_TILES = [(0, 128, 0, 16), (128, 128, 32, 16), (256, 64, 64, 8)]
# (psum_row_start, row_count, out_channel_start) for the gwc output DMAs
GWC_BLOCKS = [(0, 16, 0), (32, 16, 16), (64, 8, 32)]
PSUM_P = 72  # psum/staging partition extent

_CACHE = {}


def _make_weights():
    """Block-diagonal mean weights, bf16: W[p, m] = 1/8 if p//8 == m."""
    ws = []
    for _, cn, _, mn in CH_TILES:
        w = np.zeros((cn, mn), dtype=np.float32)
        for p in range(cn):
            w[p, p // CPG] = 1.0 / CPG
        ws.append(w.astype(ml_dtypes.bfloat16))
    return ws


def _build_nc():
    from concourse import bacc, mybir
    import concourse.tile as tile

    f32 = mybir.dt.float32
    bf16 = mybir.dt.bfloat16

    nc = bacc.Bacc("TRN2", target_bir_lowering=False, debug=False)

    ref = nc.dram_tensor("ref_gwc", [C, HS, W], f32, kind="ExternalInput")
    tgt = nc.dram_tensor("tgt_gwc", [C, HS, W], f32, kind="ExternalInput")
    refc = nc.dram_tensor("ref_concat", [CC, HS, W], f32, kind="ExternalInput")
    tgtc = nc.dram_tensor("tgt_concat", [CC, HS, W], f32, kind="ExternalInput")
    wd = [
        nc.dram_tensor(f"w{t}", [cn, mn], bf16, kind="ExternalInput")
        for t, (_, cn, _, mn) in enumerate(CH_TILES)
    ]
    out = nc.dram_tensor("out", [COUT, D, HS, W], f32, kind="ExternalOutput")

    with tile.TileContext(nc) as tc:
        _kernel_body(nc, tc, ref, tgt, refc, tgtc, wd, out, mybir)

    nc.compile()
    return nc


def _kernel_body(nc, tc, ref, tgt, refc, tgtc, wd, out, mybir):
    f32 = mybir.dt.float32
    bf16 = mybir.dt.bfloat16
    out_ap = out.ap()

    with (
        tc.tile_pool(name="const", bufs=1) as constp,
        tc.tile_pool(name="prod", bufs=2) as prodp,
        tc.tile_pool(name="psum", bufs=2, space="PSUM") as psump,
    ):
        # --- constants / persistent buffers ---
        wt = []
        for t, (_, cn, _, mn) in enumerate(CH_TILES):
            w_t = constp.tile([cn, mn], bf16, name=f"wt{t}", tag=f"wt{t}")
            nc.sync.dma_start(w_t[:], wd[t].ap())
            wt.append(w_t)

        # concat inputs (fp32, kept in SBUF, DMA'd out per-d)
        refc_t = constp.tile([CC, HS, W], f32, name="refc_t", tag="refc_t")
        nc.sync.dma_start(refc_t[:], refc.ap())
        tgtc_t = constp.tile([CC, HS, W], f32, name="tgtc_t", tag="tgtc_t")
        nc.sync.dma_start(tgtc_t[:], tgtc.ap())

        # gwc inputs as bf16 (cast happens inside the SWDGE DMA).
        # refB holds ref shifted by one element (data at [:, :, 1:W+1],
        # row stride W+4) so odd-d views stay 4-byte aligned for DVE 2x.
        # refB is derived on-chip (ACT copy) instead of re-reading HBM.
        refA, refB, tgtT = [], [], []
        for t, (c0, cn, _, _) in enumerate(CH_TILES):
            a = constp.tile([cn, HS, W], bf16, name=f"refA{t}", tag=f"refA{t}")
            nc.gpsimd.dma_start(a[:], ref[c0:c0 + cn])
            b = constp.tile([cn, HS, W + 4], bf16, name=f"refB{t}", tag=f"refB{t}")
            nc.scalar.copy(b[:, :, 1:W + 1], a[:])
            g = constp.tile([cn, HS, W], bf16, name=f"tgtT{t}", tag=f"tgtT{t}")
            nc.gpsimd.dma_start(g[:], tgt[c0:c0 + cn])
            refA.append(a)
            refB.append(b)
            tgtT.append(g)

        # concat-channel zero strips (w < d), written as 6 rectangular
        # blocks of 8 disparities each; the per-d data DMAs re-cover
        # [d:47] afterwards (explicit dep edges enforce the order).
        zrect = constp.tile([2 * CC, 4, HS, D - 1], f32, name="zrect",
                            tag="zrect")
        nc.gpsimd.memset(zrect[:], 0.0)
        zrect_inst = {}
        for blk in range(12):
            d0 = 1 + 4 * blk
            nd = min(4, D - d0)
            inst = nc.scalar.dma_start(
                out_ap[G:COUT, d0:d0 + nd, :, 0:D - 1],
                zrect[:, 0:nd, :, :])
            for dd in range(d0, d0 + nd):
                zrect_inst[dd] = inst

        # staging buffers (manual 3-slot rotation; zeroed once, then the
        # descending-d order keeps the w<d strip zero forever)
        stg = []
        for i in range(3):
            s = constp.tile([PSUM_P, HS, W], f32, name=f"stg{i}", tag=f"stg{i}")
            nc.vector.memset(s[:], 0.0)
            stg.append(s)

        # --- main disparity loop (descending) ---
        for di, d in enumerate(reversed(range(D))):
            wv = W - d
            s = stg[di % 3]

            # products (bf16) on DVE
            prods = []
            for t, (_, cn, _, _) in enumerate(CH_TILES):
                p = prodp.tile([cn, HS, W], bf16, name=f"prod{t}_{d}",
                               tag=f"prod{t}")
                if d % 2 == 0:
                    rsrc = refA[t][0:cn, :, d:W]
                else:
                    rsrc = refB[t][0:cn, :, d + 1:W + 1]
                nc.vector.tensor_mul(p[0:cn, :, 0:wv], rsrc,
                                     tgtT[t][0:cn, :, 0:wv])
                prods.append(p)

            # group-reduce on PE, drain on ACT, one h-half at a time
            for hh in range(2):
                ps = psump.tile([PSUM_P, HS // 2, 256], f32,
                                name=f"ps_{d}_{hh}", tag="ps")
                for t, (_, cn, m0, mn) in enumerate(CH_TILES):
                    for k in range(4):
                        h0 = hh * 8 + 2 * k
                        nc.tensor.matmul(
                            ps[m0:m0 + mn, 2 * k:2 * k + 2, d:W],
                            wt[t][0:cn, 0:mn],
                            prods[t][0:cn, h0:h0 + 2, 0:wv],
                            start=True, stop=True,
                        )
                nc.scalar.copy(s[:, hh * 8:hh * 8 + 8, d:W], ps[:, :, d:W])

            # gwc channels out (full width; w<d strip is already zero)
            for p0, pn, c0 in GWC_BLOCKS:
                nc.sync.dma_start(out_ap[c0:c0 + pn, d], s[p0:p0 + pn])

            # concat channels straight from SBUF (split across the two
            # HWDGE rings; each must run after its covering zero-rect)
            i1 = nc.scalar.dma_start(out_ap[G:G + CC, d, :, d:W],
                                     refc_t[:, :, d:W])
            i2 = nc.sync.dma_start(out_ap[G + CC:COUT, d, :, d:W],
                                   tgtc_t[:, :, 0:wv])
            if d in zrect_inst:
                from concourse.bass import _add_dep_helper
                _add_dep_helper(i1.ins, zrect_inst[d].ins, sync=True,
                                reason="concat data after zero-rect")
                _add_dep_helper(i2.ins, zrect_inst[d].ins, sync=True,
                                reason="concat data after zero-rect")


def _get_nc():
    if "nc" not in _CACHE:
        _CACHE["nc"] = _build_nc()
    return _CACHE["nc"]


def kernel(ref_gwc, tgt_gwc, ref_concat, tgt_concat):
    from concourse.bass_utils import run_bass_kernel_spmd

    ref_gwc = np.asarray(ref_gwc, dtype=np.float32)
    tgt_gwc = np.asarray(tgt_gwc, dtype=np.float32)
    ref_concat = np.asarray(ref_concat, dtype=np.float32)
    tgt_concat = np.asarray(tgt_concat, dtype=np.float32)

    nc = _get_nc()
    ws = _make_weights()

    in_maps = []
    for i in range(NCORES):
        sl = slice(i * HS, (i + 1) * HS)
        m = {
            "ref_gwc": np.ascontiguousarray(ref_gwc[0, :, sl, :]),
            "tgt_gwc": np.ascontiguousarray(tgt_gwc[0, :, sl, :]),
            "ref_concat": np.ascontiguousarray(ref_concat[0, :, sl, :]),
            "tgt_concat": np.ascontiguousarray(tgt_concat[0, :, sl, :]),
        }
        for t, w in enumerate(ws):
            m[f"w{t}"] = w
        in_maps.append(m)

    res = run_bass_kernel_spmd(nc, in_maps, list(range(NCORES))).results

    full = np.empty((1, COUT, D, H, W), dtype=np.float32)
    for i in range(NCORES):
        full[0, :, :, i * HS:(i + 1) * HS, :] = res[i]["out"]
    return full



# revision 6
# speedup vs baseline: 1.0700x; 1.0700x over previous
"""GwcVolumeCostProcessor Trainium2 kernel (v2: DMA-restructured).

Builds the groupwise-correlation + concat cost volume:
  out[1, 64, 48, 128, 240] f32 from
  ref_gwc/tgt_gwc [1, 320, 128, 240] and ref_concat/tgt_concat [1, 12, 128, 240].

Sharding: H axis (128 = 8 cores x 16 rows). The disparity shift is along W
only, so each core needs just its own 16-row slice of every input.

Per-core pipeline (for each disparity d, descending):
  - DVE: prod[c,h,w] = ref_bf16[c,h,w+d] * tgt_bf16[c,h,w]   (bf16, 2x mode)
  - PE : block-diagonal [128,16] bf16 matmul reduces groups of 8 channels
         (mean) into PSUM
  - ACT: drains PSUM -> d-chunked gwc staging (fp32); also builds the
         concat staging (masked ref / shifted tgt) on 96 packed partitions
  - DMA: large d-chunked stores (~0.5-1.8MB per dma_start, 30KB+
         descriptors) spread across the sync/scalar HWDGE queues and the
         gpsimd SWDGE queue.

Output DRAM layouts are per-core custom (gwc: [40,48,16,240]; concat:
[96,48,2,240] with partition=(channel, h-pair)); the host reassembles the
full [1,64,48,128,240] tensor with pure layout transforms.

Staging slots are zeroed once; descending-d order keeps the w<d strip zero
without per-d memsets (each reuse of a slot covers a superset of the
previously written columns).
"""

import numpy as np
import ml_dtypes

C = 320          # gwc channels
G = 40           # groups
CPG = 8          # channels per group
D = 48           # disparity bins
H = 128          # full height
W = 240          # width
CC = 12          # concat channels per tensor
COUT = G + 2 * CC  # 64 output channels
NCORES = 8
HS = H // NCORES  # 16 rows per core

# channel tiles on partitions: (start, count, psum_base_partition, out_group_count)
# psum base partitions must be 32-aligned (PE col_grp constraint), so the
# three group blocks land sparsely at psum/staging partitions 0, 32, 64.
CH_TILES = [(0, 128, 0, 16), (128, 128, 32, 16), (256, 64, 64, 8)]
# (psum_row_start, row_count, out_channel_start) for the gwc output DMAs
GWC_BLOCKS = [(0, 16, 0), (32, 16, 16), (64, 8, 32)]
PSUM_P = 72   # psum/staging partition extent
DN = 2        # disparities per gwc staging chunk
DNC = 4       # disparities per concat staging chunk
CCP = 2 * CC * HS // 2 // 2  # 96 packed partitions for concat: (c, h-pair)

_CACHE = {}


def _make_weights():
    """Block-diagonal mean weights, bf16: W[p, m] = 1/8 if p//8 == m."""
    ws = []
    for _, cn, _, mn in CH_TILES:
        w = np.zeros((cn, mn), dtype=np.float32)
        for p in range(cn):
            w[p, p // CPG] = 1.0 / CPG
        ws.append(w.astype(ml_dtypes.bfloat16))
    return ws


def _build_nc():
    from concourse import bacc, mybir
    import concourse.tile as tile

    f32 = mybir.dt.float32
    bf16 = mybir.dt.bfloat16

    nc = bacc.Bacc("TRN2", target_bir_lowering=False, debug=False)

    ref = nc.dram_tensor("ref_gwc", [C, HS, W], f32, kind="ExternalInput")
    tgt = nc.dram_tensor("tgt_gwc", [C, HS, W], f32, kind="ExternalInput")
    # concat inputs pre-packed on host: partition = (c, h/2), rows of 2*W
    refc = nc.dram_tensor("ref_concat", [96, 2, W], f32, kind="ExternalInput")
    tgtc = nc.dram_tensor("tgt_concat", [96, 2, W], f32, kind="ExternalInput")
    wd = [
        nc.dram_tensor(f"w{t}", [cn, mn], bf16, kind="ExternalInput")
        for t, (_, cn, _, mn) in enumerate(CH_TILES)
    ]
    og = nc.dram_tensor("og", [G, D, HS, W], f32, kind="ExternalOutput")
    ocr = nc.dram_tensor("ocr", [96, D, 2, W], f32, kind="ExternalOutput")
    oct_ = nc.dram_tensor("oct", [96, D, 2, W], f32, kind="ExternalOutput")

    with tile.TileContext(nc) as tc:
        _kernel_body(nc, tc, ref, tgt, refc, tgtc, wd, og, ocr, oct_, mybir)

    nc.compile()
    return nc


def _kernel_body(nc, tc, ref, tgt, refc, tgtc, wd, og, ocr, oct_, mybir):
    f32 = mybir.dt.float32
    bf16 = mybir.dt.bfloat16
    og_ap = og.ap()
    ocr_ap = ocr.ap()
    oct_ap = oct_.ap()

    with (
        tc.tile_pool(name="const", bufs=1) as constp,
        tc.tile_pool(name="prod", bufs=3) as prodp,
        tc.tile_pool(name="psum", bufs=2, space="PSUM") as psump,
    ):
        # --- constants / persistent buffers ---
        wt = []
        for t, (_, cn, _, mn) in enumerate(CH_TILES):
            w_t = constp.tile([cn, mn], bf16, name=f"wt{t}", tag=f"wt{t}")
            nc.sync.dma_start(w_t[:], wd[t].ap())
            wt.append(w_t)

        # concat inputs, packed [(c,h2)=96, 2, W] f32, kept in SBUF
        refc_t = constp.tile([96, 2, W], f32, name="refc_t", tag="refc_t")
        nc.sync.dma_start(refc_t[:], refc.ap())
        tgtc_t = constp.tile([96, 2, W], f32, name="tgtc_t", tag="tgtc_t")
        nc.scalar.dma_start(tgtc_t[:], tgtc.ap())

        # gwc inputs as bf16 (cast happens inside the SWDGE DMA).
        # refB holds ref shifted by one element (data at [:, :, 1:W+1],
        # row stride W+4) so odd-d views stay 4-byte aligned for DVE 2x.
        # refB is derived on-chip (ACT copy) instead of re-reading HBM.
        refA, refB, tgtT = [], [], []
        for t, (c0, cn, _, _) in enumerate(CH_TILES):
            a = constp.tile([cn, HS, W], bf16, name=f"refA{t}", tag=f"refA{t}")
            nc.gpsimd.dma_start(a[:], ref[c0:c0 + cn])
            b = constp.tile([cn, HS, W + 4], bf16, name=f"refB{t}", tag=f"refB{t}")
            nc.scalar.copy(b[:, :, 1:W + 1], a[:])
            g = constp.tile([cn, HS, W], bf16, name=f"tgtT{t}", tag=f"tgtT{t}")
            nc.gpsimd.dma_start(g[:], tgt[c0:c0 + cn])
            refA.append(a)
            refB.append(b)
            tgtT.append(g)

        # gwc staging: [72, DN, 16, 240] f32, zeroed once (descending-d
        # reuse keeps the w<d strip zero forever).  Manual 2-slot rotation.
        stg = []
        for i in range(2):
            s = constp.tile([PSUM_P, DN, HS, W], f32, name=f"stg{i}",
                          tag=f"stg{i}")
            nc.vector.memset(s[:], 0.0)
            stg.append(s)

        # concat staging: [(c,h2)=96, DNC, 2, 240] f32 per side, zeroed
        # once, manual 2-slot rotation.
        ccs = []
        for i in range(2):
            sr = constp.tile([96, DNC, 2, W], f32, name=f"ccr{i}", tag=f"ccr{i}")
            nc.gpsimd.memset(sr[:], 0.0)
            st = constp.tile([96, DNC, 2, W], f32, name=f"cct{i}", tag=f"cct{i}")
            nc.gpsimd.memset(st[:], 0.0)
            ccs.append((sr, st))

        # HWDGE byte-balancing round robin
        hw_q = [nc.sync, nc.scalar]
        qbytes = [0, 0, 0]  # sync, scalar, gpsimd

        def pick_hw():
            i = 0 if qbytes[0] <= qbytes[1] else 1
            return i

        def issue(eng_idx, dst, src, nbytes):
            qbytes[eng_idx] += nbytes
            if eng_idx == 2:
                return nc.gpsimd.dma_start(dst, src)
            return hw_q[eng_idx].dma_start(dst, src)

        # --- main disparity loop (descending chunks) ---
        # gwc chunks of DN=2, concat chunks of DNC=4
        for kc in reversed(range(D // DNC)):
            # concat staging tiles for this chunk (manual rotation)
            ccr, cct = ccs[kc % 2]

            for kg in reversed(range(kc * DNC // DN, (kc + 1) * DNC // DN)):
                s = stg[kg % 2]
                for i in reversed(range(DN)):
                    d = kg * DN + i
                    wv = W - d

                    # products (bf16) on DVE
                    prods = []
                    for t, (_, cn, _, _) in enumerate(CH_TILES):
                        p = prodp.tile([cn, HS, W], bf16, name=f"prod{t}_{d}",
                                       tag="prod")
                        if d % 2 == 0:
                            rsrc = refA[t][0:cn, :, d:W]
                        else:
                            rsrc = refB[t][0:cn, :, d + 1:W + 1]
                        nc.vector.tensor_mul(p[0:cn, :, 0:wv], rsrc,
                                             tgtT[t][0:cn, :, 0:wv])
                        prods.append(p)

                    # group-reduce on PE, drain on ACT, one h-half at a time
                    for hh in range(2):
                        ps = psump.tile([PSUM_P, HS // 2, 256], f32,
                                        name=f"ps_{d}_{hh}", tag="ps")
                        for t, (_, cn, m0, mn) in enumerate(CH_TILES):
                            for k in range(4):
                                h0 = hh * 8 + 2 * k
                                nc.tensor.matmul(
                                    ps[m0:m0 + mn, 2 * k:2 * k + 2, d:W],
                                    wt[t][0:cn, 0:mn],
                                    prods[t][0:cn, h0:h0 + 2, 0:wv],
                                    start=True, stop=True,
                                )
                        nc.scalar.copy(s[:, i, hh * 8:hh * 8 + 8, d:W],
                                       ps[:, :, d:W])

                    # concat staging copies (ACT): masked ref / shifted tgt
                    ic = d - kc * DNC
                    nc.scalar.copy(ccr[:, ic, :, d:W], refc_t[:, :, d:W])
                    nc.scalar.copy(cct[:, ic, :, d:W], tgtc_t[:, :, 0:wv])

                # gwc chunk out: 3 sparse blocks, big descriptors
                d0 = kg * DN
                for p0, pn, c0 in GWC_BLOCKS:
                    nb = pn * DN * HS * W * 4
                    issue(pick_hw(), og_ap[c0:c0 + pn, d0:d0 + DN],
                          s[p0:p0 + pn], nb)

            # concat chunk out
            d0 = kc * DNC
            nb = 96 * DNC * 2 * W * 4
            # spread: give SWDGE a share of the concat stores
            if kc % 3 == 2:
                issue(2, ocr_ap[:, d0:d0 + DNC], ccr[:], nb)
                issue(pick_hw(), oct_ap[:, d0:d0 + DNC], cct[:], nb)
            elif kc % 3 == 1:
                issue(pick_hw(), ocr_ap[:, d0:d0 + DNC], ccr[:], nb)
                issue(2, oct_ap[:, d0:d0 + DNC], cct[:], nb)
            else:
                issue(pick_hw(), ocr_ap[:, d0:d0 + DNC], ccr[:], nb)
                issue(pick_hw(), oct_ap[:, d0:d0 + DNC], cct[:], nb)


def _get_nc():
    if "nc" not in _CACHE:
        _CACHE["nc"] = _build_nc()
    return _CACHE["nc"]


def _pack_concat(x):
    # [12, 16, 240] -> [(c,h2)=96, 2, 240]
    return np.ascontiguousarray(x.reshape(CC, 8, 2, W).reshape(96, 2, W))


def kernel(ref_gwc, tgt_gwc, ref_concat, tgt_concat):
    from concourse.bass_utils import run_bass_kernel_spmd

    ref_gwc = np.asarray(ref_gwc, dtype=np.float32)
    tgt_gwc = np.asarray(tgt_gwc, dtype=np.float32)
    ref_concat = np.asarray(ref_concat, dtype=np.float32)
    tgt_concat = np.asarray(tgt_concat, dtype=np.float32)

    nc = _get_nc()
    ws = _make_weights()

    in_maps = []
    for i in range(NCORES):
        sl = slice(i * HS, (i + 1) * HS)
        m = {
            "ref_gwc": np.ascontiguousarray(ref_gwc[0, :, sl, :]),
            "tgt_gwc": np.ascontiguousarray(tgt_gwc[0, :, sl, :]),
            "ref_concat": _pack_concat(ref_concat[0, :, sl, :]),
            "tgt_concat": _pack_concat(tgt_concat[0, :, sl, :]),
        }
        for t, w in enumerate(ws):
            m[f"w{t}"] = w
        in_maps.append(m)

    res = run_bass_kernel_spmd(nc, in_maps, list(range(NCORES))).results

    full = np.empty((1, COUT, D, H, W), dtype=np.float32)
    for i in range(NCORES):
        sl = slice(i * HS, (i + 1) * HS)
        full[0, 0:G, :, sl, :] = res[i]["og"]
        # [96, 48, 2, 240] -> [12, 48, 16, 240]
        for nm, c0 in (("ocr", G), ("oct", G + CC)):
            v = res[i][nm].reshape(CC, 8, D, 2, W)
            full[0, c0:c0 + CC, :, sl, :] = (
                v.transpose(0, 2, 1, 3, 4).reshape(CC, D, HS, W))
    return full


# revision 8
# speedup vs baseline: 1.8796x; 1.7565x over previous
"""GwcVolumeCostProcessor Trainium2 kernel (v3).

Builds the groupwise-correlation + concat cost volume:
  out[1, 64, 48, 128, 240] f32 from
  ref_gwc/tgt_gwc [1, 320, 128, 240] and ref_concat/tgt_concat [1, 12, 128, 240].

Sharding: H axis (128 = 8 cores x 16 rows).

Key structure (v3):
  - Dense PSUM: groups 0-31 come from TWO accumulating matmuls (channels
    0-127 with weights W1 [128,32] and channels 128-255 with W2 [128,32],
    start/stop accumulation) -> psum partitions 0-31 dense.  Channels
    256-319 are (c,h2)-packed on 128 partitions (h_sub=2) with a
    block-diag-over-h2 weight [128,16] -> psum partitions 32-47.
  - bf16 staging + bf16 DRAM outputs (host upcasts): writes 26.6MB
    instead of 47.2MB.
  - Output DMAs span 48 / 96 partitions with multi-KB descriptors (the
    per-dma_start rate scales with partition coverage: partitions map to
    SDMA engines in blocks of 4).
  - Per-core DRAM output layouts are custom; the host reassembles with
    pure layout transforms.

Engines: DVE products (bf16 2x), PE group-mean reduce, ACT drains +
concat shift/mask staging, all three DMA queues for stores.
"""

import numpy as np
import ml_dtypes

C = 320          # gwc channels
G = 40           # groups
CPG = 8          # channels per group
D = 48           # disparity bins
H = 128          # full height
W = 240          # width
CC = 12          # concat channels per tensor
COUT = G + 2 * CC  # 64 output channels
NCORES = 8
HS = H // NCORES  # 16 rows per core

DN = 4    # disparities per gwc staging chunk
DNC = 8   # disparities per concat staging chunk
SP = 48   # staging/psum partitions (32 dense groups + 16 (g,h2) rows)

_CACHE = {}


def _make_weights():
    """W1/W2 [128,32] accumulating pair for groups 0-31; w2x [128,16]
    block-diag over (c',h2) for groups 32-39."""
    w1 = np.zeros((128, 32), dtype=np.float32)
    w2 = np.zeros((128, 32), dtype=np.float32)
    for c in range(128):
        w1[c, c // CPG] = 1.0 / CPG        # groups 0..15
        w2[c, 16 + c // CPG] = 1.0 / CPG   # groups 16..31
    w2x = np.zeros((128, 16), dtype=np.float32)
    for p in range(128):
        cp, h2 = p // 2, p % 2
        w2x[p, (cp // CPG) * 2 + h2] = 1.0 / CPG  # groups 32..39 x h2
    bf = ml_dtypes.bfloat16
    return w1.astype(bf), w2.astype(bf), w2x.astype(bf)


def _build_nc():
    from concourse import bacc, mybir
    import concourse.tile as tile

    f32 = mybir.dt.float32
    bf16 = mybir.dt.bfloat16

    nc = bacc.Bacc("TRN2", target_bir_lowering=False, debug=False)

    ref01 = nc.dram_tensor("ref01", [256, HS, W], f32, kind="ExternalInput")
    tgt01 = nc.dram_tensor("tgt01", [256, HS, W], f32, kind="ExternalInput")
    ref2 = nc.dram_tensor("ref2", [128, HS // 2, W], f32, kind="ExternalInput")
    tgt2 = nc.dram_tensor("tgt2", [128, HS // 2, W], f32, kind="ExternalInput")
    refc = nc.dram_tensor("ref_concat", [96, 2, W], f32, kind="ExternalInput")
    tgtc = nc.dram_tensor("tgt_concat", [96, 2, W], f32, kind="ExternalInput")
    w1d = nc.dram_tensor("w1", [128, 32], bf16, kind="ExternalInput")
    w2d = nc.dram_tensor("w2", [128, 32], bf16, kind="ExternalInput")
    w2xd = nc.dram_tensor("w2x", [128, 16], bf16, kind="ExternalInput")
    og = nc.dram_tensor("og", [SP, D, HS, W], bf16, kind="ExternalOutput")
    ocr = nc.dram_tensor("ocr", [96, D, 2, W], bf16, kind="ExternalOutput")
    oct_ = nc.dram_tensor("oct", [96, D, 2, W], bf16, kind="ExternalOutput")

    with tile.TileContext(nc) as tc:
        _kernel_body(nc, tc, ref01, tgt01, ref2, tgt2, refc, tgtc,
                     (w1d, w2d, w2xd), og, ocr, oct_, mybir)

    nc.compile()
    return nc


def _kernel_body(nc, tc, ref01, tgt01, ref2, tgt2, refc, tgtc, wds,
                 og, ocr, oct_, mybir):
    f32 = mybir.dt.float32
    bf16 = mybir.dt.bfloat16
    og_ap = og.ap()
    ocr_ap = ocr.ap()
    oct_ap = oct_.ap()
    w1d, w2d, w2xd = wds

    with (
        tc.tile_pool(name="const", bufs=1) as constp,
        tc.tile_pool(name="prod", bufs=4) as prodp,
        tc.tile_pool(name="psum", bufs=2, space="PSUM") as psump,
    ):
        # --- weights ---
        w1 = constp.tile([128, 32], bf16, name="w1t", tag="w1t")
        nc.sync.dma_start(w1[:], w1d.ap())
        w2 = constp.tile([128, 32], bf16, name="w2t", tag="w2t")
        nc.sync.dma_start(w2[:], w2d.ap())
        w2x = constp.tile([128, 16], bf16, name="w2xt", tag="w2xt")
        nc.sync.dma_start(w2x[:], w2xd.ap())

        # --- concat inputs (bf16, packed [(c,h8)=96, 2, W]), on
        # partitions 32..127 so concat store-DMAs cover all 16 engines ---
        refc_t = constp.tile([96, 2, W], bf16, name="refc_t", tag="refc_t")
        nc.gpsimd.dma_start(refc_t[:], refc.ap())
        tgtc_t = constp.tile([96, 2, W], bf16, name="tgtc_t", tag="tgtc_t")
        nc.gpsimd.dma_start(tgtc_t[:], tgtc.ap())

        # --- gwc inputs as bf16 (cast inside SWDGE DMA) ---
        # tiles 0/1: [128, 16, W] channels 0-127 / 128-255
        # tile 2: (c',h2)-packed [128, 8, W] channels 256-319
        # refB* shifted by one element for odd-d 4B alignment (DVE 2x)
        refA, refB, tgtT = [], [], []
        for t in range(2):
            a = constp.tile([128, HS, W], bf16, name=f"refA{t}", tag=f"refA{t}")
            nc.gpsimd.dma_start(a[:], ref01[128 * t:128 * (t + 1)])
            b = constp.tile([128, HS, W + 4], bf16, name=f"refB{t}",
                            tag=f"refB{t}")
            nc.scalar.copy(b[:, :, 1:W + 1], a[:])
            g = constp.tile([128, HS, W], bf16, name=f"tgtT{t}", tag=f"tgtT{t}")
            nc.gpsimd.dma_start(g[:], tgt01[128 * t:128 * (t + 1)])
            refA.append(a)
            refB.append(b)
            tgtT.append(g)
        a2 = constp.tile([128, HS // 2, W], bf16, name="refA2", tag="refA2")
        nc.gpsimd.dma_start(a2[:], ref2.ap())
        b2 = constp.tile([128, HS // 2, W + 4], bf16, name="refB2", tag="refB2")
        nc.scalar.copy(b2[:, :, 1:W + 1], a2[:])
        g2 = constp.tile([128, HS // 2, W], bf16, name="tgtT2", tag="tgtT2")
        nc.gpsimd.dma_start(g2[:], tgt2.ap())

        # --- staging, zeroed once; manual 2-slot rotation; descending-d
        # reuse keeps the w<d strip zero ---
        stg = []
        for i in range(2):
            s = constp.tile([SP, DN, HS, W], bf16, name=f"stg{i}",
                            tag=f"stg{i}")
            nc.gpsimd.memset(s[:], 0.0)
            stg.append(s)
        ccs = []
        for i in range(2):
            sr = constp.tile([96, DNC, 2, W], bf16, name=f"ccr{i}",
                             tag=f"ccr{i}")
            nc.gpsimd.memset(sr[:], 0.0)
            st = constp.tile([96, DNC, 2, W], bf16, name=f"cct{i}",
                             tag=f"cct{i}")
            nc.gpsimd.memset(st[:], 0.0)
            ccs.append((sr, st))

        hw_q = [nc.sync, nc.scalar]

        # --- main disparity loop (descending chunks) ---
        for kc in reversed(range(D // DNC)):
            ccr, cct = ccs[kc % 2]

            for kg in reversed(range(kc * DNC // DN, (kc + 1) * DNC // DN)):
                s = stg[kg % 2]
                for i in reversed(range(DN)):
                    d = kg * DN + i
                    wv = W - d

                    # products (bf16) on DVE: t0, t1 [128,16,wv]; t2 [128,8,wv]
                    prods = []
                    for t in range(2):
                        p = prodp.tile([128, HS, W], bf16,
                                       name=f"prod{t}_{d}", tag="prod")
                        if d % 2 == 0:
                            rsrc = refA[t][:, :, d:W]
                        else:
                            rsrc = refB[t][:, :, d + 1:W + 1]
                        nc.vector.tensor_mul(p[:, :, 0:wv], rsrc,
                                             tgtT[t][:, :, 0:wv])
                        prods.append(p)
                    p2 = prodp.tile([128, HS, W], bf16, name=f"prod2_{d}",
                                    tag="prod")
                    if d % 2 == 0:
                        rsrc2 = a2[:, :, d:W]
                    else:
                        rsrc2 = b2[:, :, d + 1:W + 1]
                    nc.vector.tensor_mul(p2[:, 0:HS // 2, 0:wv], rsrc2,
                                         g2[:, :, 0:wv])

                    # group-reduce on PE -> dense psum [48, 8, wv]
                    # hh0: groups 0-31 (h 0-7) + (g,h2) rows 32-47 (all h)
                    # hh1: groups 0-31 (h 8-15)
                    for hh in range(2):
                        ps = psump.tile([SP, HS // 2, 256], f32,
                                        name=f"ps_{d}_{hh}", tag="ps")
                        for k in range(4):
                            h0 = hh * 8 + 2 * k
                            nc.tensor.matmul(
                                ps[0:32, 2 * k:2 * k + 2, d:W],
                                w1[:, :],
                                prods[0][:, h0:h0 + 2, 0:wv],
                                start=True, stop=False,
                            )
                            nc.tensor.matmul(
                                ps[0:32, 2 * k:2 * k + 2, d:W],
                                w2[:, :],
                                prods[1][:, h0:h0 + 2, 0:wv],
                                start=False, stop=True,
                            )
                            if hh == 0:
                                nc.tensor.matmul(
                                    ps[32:48, 2 * k:2 * k + 2, d:W],
                                    w2x[:, :],
                                    p2[:, 2 * k:2 * k + 2, 0:wv],
                                    start=True, stop=True,
                                )
                        if hh == 0:
                            nc.scalar.copy(s[0:SP, i, 0:8, d:W],
                                           ps[0:SP, :, d:W])
                        else:
                            nc.scalar.copy(s[0:32, i, 8:16, d:W],
                                           ps[0:32, :, d:W])

                    # concat staging copies (ACT, bf16 2x)
                    ic = d - kc * DNC
                    nc.scalar.copy(ccr[:, ic, :, d:W],
                                   refc_t[:, :, d:W])
                    nc.scalar.copy(cct[:, ic, :, d:W],
                                   tgtc_t[:, :, 0:wv])

                # gwc chunk out: one 48-partition store
                d0 = kg * DN
                hw_q[kg % 2].dma_start(og_ap[:, d0:d0 + DN], s[:])

            # concat chunk out: 96-partition stores (all 16 engines)
            d0 = kc * DNC
            if kc % 3 == 2:
                nc.gpsimd.dma_start(ocr_ap[:, d0:d0 + DNC], ccr[:])
                hw_q[kc % 2].dma_start(oct_ap[:, d0:d0 + DNC], cct[:])
            elif kc % 3 == 1:
                hw_q[kc % 2].dma_start(ocr_ap[:, d0:d0 + DNC], ccr[:])
                nc.gpsimd.dma_start(oct_ap[:, d0:d0 + DNC], cct[:])
            else:
                hw_q[0].dma_start(ocr_ap[:, d0:d0 + DNC], ccr[:])
                hw_q[1].dma_start(oct_ap[:, d0:d0 + DNC], cct[:])


def _get_nc():
    if "nc" not in _CACHE:
        _CACHE["nc"] = _build_nc()
    return _CACHE["nc"]


def _pack_concat(x):
    # [12, 16, 240] -> [(c,h8)=96, 2, 240]
    return np.ascontiguousarray(x.reshape(96, 2, W))


def _make_in_map(ref_gwc, tgt_gwc, ref_concat, tgt_concat, ws):
    w1, w2, w2x = ws
    return {
        "ref01": np.ascontiguousarray(ref_gwc[0:256]),
        "tgt01": np.ascontiguousarray(tgt_gwc[0:256]),
        # channels 256-319 (c',h2)-packed: [64,16,240]->[128,8,240]
        "ref2": np.ascontiguousarray(ref_gwc[256:320].reshape(128, 8, W)),
        "tgt2": np.ascontiguousarray(tgt_gwc[256:320].reshape(128, 8, W)),
        "ref_concat": _pack_concat(ref_concat),
        "tgt_concat": _pack_concat(tgt_concat),
        "w1": w1, "w2": w2, "w2x": w2x,
    }


def kernel(ref_gwc, tgt_gwc, ref_concat, tgt_concat):
    from concourse.bass_utils import run_bass_kernel_spmd

    ref_gwc = np.asarray(ref_gwc, dtype=np.float32)
    tgt_gwc = np.asarray(tgt_gwc, dtype=np.float32)
    ref_concat = np.asarray(ref_concat, dtype=np.float32)
    tgt_concat = np.asarray(tgt_concat, dtype=np.float32)

    nc = _get_nc()
    ws = _make_weights()

    in_maps = []
    for i in range(NCORES):
        sl = slice(i * HS, (i + 1) * HS)
        in_maps.append(_make_in_map(ref_gwc[0, :, sl, :], tgt_gwc[0, :, sl, :],
                                    ref_concat[0, :, sl, :],
                                    tgt_concat[0, :, sl, :], ws))

    res = run_bass_kernel_spmd(nc, in_maps, list(range(NCORES))).results

    full = np.empty((1, COUT, D, H, W), dtype=np.float32)
    for i in range(NCORES):
        sl = slice(i * HS, (i + 1) * HS)
        ogv = np.asarray(res[i]["og"]).astype(np.float32)
        full[0, 0:32, :, sl, :] = ogv[0:32]
        # rows 32-47: (g,h2) x [d, h'(8), w]
        t2 = ogv[32:48, :, 0:8, :].reshape(8, 2, D, 8, W)
        full[0, 32:40, :, sl, :] = (
            t2.transpose(0, 2, 1, 3, 4).reshape(8, D, HS, W))
        # [96, 48, 2, 240] -> [12, 48, 16, 240]
        for nm, c0 in (("ocr", G), ("oct", G + CC)):
            v = np.asarray(res[i][nm]).astype(np.float32)
            v = v.reshape(CC, 8, D, 2, W)
            full[0, c0:c0 + CC, :, sl, :] = (
                v.transpose(0, 2, 1, 3, 4).reshape(CC, D, HS, W))
    return full
